# revision 1
# baseline (speedup 1.0000x reference)
"""AverageNode2Vec loss on 8 Trainium2 NeuronCores.

Strategy: data-parallel over the batch (1024 rows/core); both embedding
tables are replicated in each core's HBM as one combined [2M, 128] f32
table (v rows at +1M). Each partition owns 8 batch rows laid out as 56
gather slots (8 u + 8 v + 40 neg). Ten indirect-DMA gathers (one per
sequence position l) fetch 56 rows per partition into three rotating
SBUF staging regions; the DVE folds (adds) them into one accumulator,
computing the length-masked embedding sums. Padding indices point at
table row 0, which the host zeroes, so masking is free. A short DVE/ACT
epilogue forms the u.v dots, negative-sample scores, softplus terms
(Exp+Ln, since Softplus is absent from this compiler's ACT tables) and
per-partition partial sums; the host adds 8x128x2 partials, divides by B.

HW contract notes (verified by probes in this repo):
 - indirect_dma_start needs a flat 2-dim dst AP [128, n*D]; idx [128, n]
   int32 pairs p-major: dst[p, j*D:(j+1)*D] = table[idx[p, j]].
 - CCE compute_op on the indirect path is device-fatal; plain gathers +
   DVE folds instead.
 - This walrus build allows at most ONE attached sync wait per
   instruction; raw bass with explicit wait_ge instructions sidesteps it.
 - Measured steady-state: ~74 us per iteration on 8 cores (repeat-slope
   method; ~66 us is pure gather DMA at ~306 GB/s/core of random rows).
"""
import os
import numpy as np

VOCAB = 1_000_000
D = 128
B = 8192
NEG = 5
L = 10
NCORES = 8
BPC = B // NCORES            # batch rows per core
T = BPC // 128               # tile slots per partition (8)
SLOTS = T + T + T * NEG      # 56 gather slots per partition
OOB = 2 * VOCAB + 64         # skipped by bounds check

_STATE = {}
LAST_EXEC_NS = None


def _build_program(repeat=1, bf16=False):
    import concourse.bass as bass
    import concourse.mybir as mybir
    from concourse.bass import AP, IndirectOffsetOnAxis

    f32, i32 = mybir.dt.float32, mybir.dt.int32
    tdt = mybir.dt.bfloat16 if bf16 else f32
    nc = bass.Bass()
    big = nc.dram_tensor("big", [2 * VOCAB, D], tdt, kind="ExternalInput")
    idx_in = nc.dram_tensor("idx", [128, L * SLOTS], i32, kind="ExternalInput")
    ruv_in = nc.dram_tensor("ruv", [128, T], f32, kind="ExternalInput")
    rn_in = nc.dram_tensor("rn", [128, T * NEG], f32, kind="ExternalInput")
    out = nc.dram_tensor("lp", [128, 2], f32, kind="ExternalOutput")

    NU = T * D               # 1024: u block columns in acc
    NN = T * NEG * D         # 5120: neg block columns

    from contextlib import ExitStack
    ctx = ExitStack()
    with ctx:
        idx_t = ctx.enter_context(nc.sbuf_tensor([128, L * SLOTS], i32))
        g_a = ctx.enter_context(nc.sbuf_tensor([128, SLOTS * D], tdt))
        g_b = ctx.enter_context(nc.sbuf_tensor([128, SLOTS * D], tdt))
        g_c = ctx.enter_context(nc.sbuf_tensor([128, SLOTS * D], tdt))
        acc = ctx.enter_context(nc.sbuf_tensor([128, SLOTS * D], tdt))
        ruv_t = ctx.enter_context(nc.sbuf_tensor([128, T], f32))
        rn_t = ctx.enter_context(nc.sbuf_tensor([128, T * NEG], f32))
        prod_uv = ctx.enter_context(nc.sbuf_tensor([128, NU], f32))
        prod_nu = ctx.enter_context(nc.sbuf_tensor([128, NN], f32))
        score_raw = ctx.enter_context(nc.sbuf_tensor([128, T], f32))
        nscore_raw = ctx.enter_context(nc.sbuf_tensor([128, T * NEG], f32))
        score = ctx.enter_context(nc.sbuf_tensor([128, T], f32))
        nscore = ctx.enter_context(nc.sbuf_tensor([128, T * NEG], f32))
        plt_t = ctx.enter_context(nc.sbuf_tensor([128, T], f32))
        nlt_t = ctx.enter_context(nc.sbuf_tensor([128, T * NEG], f32))
        lp_t = ctx.enter_context(nc.sbuf_tensor([128, 2], f32))
        s_idx = ctx.enter_context(nc.semaphore("s_idx"))
        s_rcp = ctx.enter_context(nc.semaphore("s_rcp"))
        s_g = ctx.enter_context(nc.semaphore("s_g"))
        s_f = ctx.enter_context(nc.semaphore("s_f"))
        s_dve = ctx.enter_context(nc.semaphore("s_dve"))
        s_act = ctx.enter_context(nc.semaphore("s_act"))
        s_out = ctx.enter_context(nc.semaphore("s_out"))
        block = ctx.enter_context(nc.Block())

        @block.sync
        def _(sync):
            sync.dma_start(out=idx_t[:], in_=idx_in[:]).then_inc(s_idx, 16)
            sync.dma_start(out=ruv_t[:], in_=ruv_in[:]).then_inc(s_rcp, 16)
            sync.dma_start(out=rn_t[:], in_=rn_in[:]).then_inc(s_rcp, 16)
            for r in range(repeat):
                sync.wait_ge(s_act, 2 * (r + 1))
                sync.dma_start(out=out[:], in_=lp_t[:]).then_inc(s_out, 16)

        regions = [g_a, g_b, g_c]

        @block.gpsimd
        def _(gpsimd):
            gpsimd.wait_ge(s_idx, 16)
            for r in range(repeat):
                for l in range(L):
                    # wait until the previous occupant of region l%3 has been
                    # consumed by its fold (see fold numbering below)
                    if r == 0:
                        tgt = max(1, l - 3) if l >= 3 else 0
                    else:
                        tgt = 9 * r + max(1, l - 3) if l >= 3 else (
                            9 * r if l == 0 else 9 * r - 3 + l)
                    if tgt > 0:
                        gpsimd.wait_ge(s_f, tgt)
                    gpsimd.indirect_dma_start(
                        out=regions[l % 3][:],
                        out_offset=None,
                        in_=big[:],
                        in_offset=IndirectOffsetOnAxis(
                            ap=idx_t[:, l * SLOTS:(l + 1) * SLOTS], axis=0
                        ),
                    ).then_inc(s_g, 16)

        def _epilogue(vector, r):
            a = acc[:]
            acc_u = acc[:, 0:NU]
            acc_v = acc[:, NU:2 * NU]
            acc_n = acc[:, 2 * NU:]
            vector.tensor_tensor(
                out=prod_uv[:], in0=acc_u, in1=acc_v, op=mybir.AluOpType.mult
            )
            vector.tensor_reduce(
                out=score_raw[:],
                in_=prod_uv[:].rearrange("p (t d) -> p t d", d=D),
                axis=mybir.AxisListType.X,
                op=mybir.AluOpType.add,
            )
            # acc_u broadcast across the NEG axis: [p, t, n(bcast), d]
            acc_u_b = AP(
                a.tensor, a.offset,
                [[a.ap[0][0], 128], [D, T], [0, NEG], [1, D]],
            )
            vector.tensor_tensor(
                out=prod_nu[:],
                in0=acc_n.rearrange("p (t n d) -> p t n d", n=NEG, d=D),
                in1=acc_u_b,
                op=mybir.AluOpType.mult,
            )
            vector.tensor_reduce(
                out=nscore_raw[:],
                in_=prod_nu[:].rearrange("p (s d) -> p s d", d=D),
                axis=mybir.AxisListType.X,
                op=mybir.AluOpType.add,
            )
            if r == 0:
                vector.wait_ge(s_rcp, 32)
            if r > 0:
                # ACT of rep r-1 must have consumed score/nscore
                vector.wait_ge(s_act, 2 * r)
            vector.tensor_tensor(
                out=score[:], in0=score_raw[:], in1=ruv_t[:],
                op=mybir.AluOpType.mult,
            ).then_inc(s_dve, 1)
            vector.tensor_tensor(
                out=nscore[:], in0=nscore_raw[:], in1=rn_t[:],
                op=mybir.AluOpType.mult,
            ).then_inc(s_dve, 1)

        @block.vector
        def _(vector):
            for r in range(repeat):
                base_g = 160 * r
                # fold 1: acc = g0 + g1; fold k (2..9): acc += region(k % 3)
                vector.wait_ge(s_g, base_g + 32)
                vector.tensor_tensor(
                    out=acc[:], in0=g_a[:], in1=g_b[:], op=mybir.AluOpType.add
                ).then_inc(s_f, 1)
                for k in range(2, L):
                    vector.wait_ge(s_g, base_g + 16 * (k + 1))
                    vector.tensor_tensor(
                        out=acc[:], in0=acc[:], in1=regions[k % 3][:],
                        op=mybir.AluOpType.add,
                    ).then_inc(s_f, 1)
                _epilogue(vector, r)

        @block.scalar
        def _(scalar):
            # softplus(x) = ln(1 + exp(x)); Softplus itself is absent from
            # this compiler's ACT table set, Exp/Ln are present.
            for r in range(repeat):
                scalar.wait_ge(s_dve, 2 * (r + 1))
                if r > 0:
                    # out-DMA of rep r-1 must have read lp_t
                    scalar.wait_ge(s_out, 16 * r)
                scalar.activation(
                    out=plt_t[:], in_=score[:],
                    func=mybir.ActivationFunctionType.Exp, scale=-1.0,
                )
                scalar.activation(
                    out=plt_t[:], in_=plt_t[:],
                    func=mybir.ActivationFunctionType.Ln,
                    bias=1.0, accum_out=lp_t[:, 0:1],
                ).then_inc(s_act, 1)
                scalar.activation(
                    out=nlt_t[:], in_=nscore[:],
                    func=mybir.ActivationFunctionType.Exp, scale=1.0,
                )
                scalar.activation(
                    out=nlt_t[:], in_=nlt_t[:],
                    func=mybir.ActivationFunctionType.Ln,
                    bias=1.0, accum_out=lp_t[:, 1:2],
                ).then_inc(s_act, 1)

    return nc


def _prep_core_inputs(c, pos_u, pos_v, neg_v, lu_all, lv_all, ln_all):
    """Index tile [128, L*SLOTS] + recip tiles for core c."""
    bsel = slice(c * BPC, (c + 1) * BPC)
    nsel = slice(c * BPC * NEG, (c + 1) * BPC * NEG)
    pu = pos_u[bsel].reshape(T, 128, L).transpose(1, 2, 0)          # [p, l, t]
    pv = pos_v[bsel].reshape(T, 128, L).transpose(1, 2, 0)          # [p, l, t]
    nv = neg_v[nsel].reshape(T, 128, NEG, L).transpose(1, 3, 0, 2)  # [p, l, t, n]
    nv = nv.reshape(128, L, T * NEG)

    idx = np.empty((128, L, SLOTS), np.int64)
    idx[:, :, 0:T] = pu
    idx[:, :, T:2 * T] = np.where(pv == 0, 0, pv + VOCAB)
    idx[:, :, 2 * T:] = np.where(nv == 0, 0, nv + VOCAB)
    # padding (index 0) gathers the zeroed row 0 and contributes nothing
    idx_flat = np.ascontiguousarray(idx.reshape(128, L * SLOTS), dtype=np.int32)

    lu = lu_all[bsel].reshape(T, 128).T.astype(np.float64)           # [p, t]
    lv = lv_all[bsel].reshape(T, 128).T.astype(np.float64)
    ln = ln_all[nsel].reshape(T, 128, NEG).transpose(1, 0, 2).astype(np.float64)
    ruv = (1.0 / (lu * lv)).astype(np.float32)
    rn = (1.0 / (lu[:, :, None] * ln)).reshape(128, T * NEG).astype(np.float32)
    return idx_flat, np.ascontiguousarray(ruv), np.ascontiguousarray(rn)


def kernel(u_table, v_table, pos_u, pos_u_lens, pos_v, pos_v_lens,
           neg_v, neg_v_lens):
    global LAST_EXEC_NS
    from concourse.bass_utils import run_bass_kernel_spmd

    u_table = np.asarray(u_table)
    v_table = np.asarray(v_table)
    pos_u = np.asarray(pos_u)
    pos_v = np.asarray(pos_v)
    neg_v = np.asarray(neg_v)
    pos_u_lens = np.asarray(pos_u_lens)
    pos_v_lens = np.asarray(pos_v_lens)
    neg_v_lens = np.asarray(neg_v_lens)

    bf16 = bool(int(os.environ.get("KERNEL_BF16", "0")))
    big = np.empty((2 * VOCAB, D), np.float32)
    big[:VOCAB] = u_table
    big[VOCAB:] = v_table
    big[0] = 0.0          # padding rows (index 0) must contribute zero
    big[VOCAB] = 0.0
    if bf16:
        import ml_dtypes
        big = big.astype(ml_dtypes.bfloat16)

    key = ("nc", bf16)
    if key not in _STATE:
        _STATE[key] = _build_program(bf16=bf16)
    nc = _STATE[key]

    in_maps = []
    for c in range(NCORES):
        idx, ruv, rn = _prep_core_inputs(
            c, pos_u, pos_v, neg_v, pos_u_lens, pos_v_lens, neg_v_lens
        )
        in_maps.append({"big": big, "idx": idx, "ruv": ruv, "rn": rn})

    trace = bool(int(os.environ.get("KERNEL_TRACE", "0")))
    res = run_bass_kernel_spmd(
        nc, in_maps, core_ids=list(range(NCORES)), trace=trace,
    )
    LAST_EXEC_NS = res.exec_time_ns

    total = np.float64(0.0)
    for r in res.results:
        total += r["lp"].astype(np.float64).sum()
    return np.float32(total / B)



# revision 7
# speedup vs baseline: 1.1741x; 1.1741x over previous
"""AverageNode2Vec loss on 8 Trainium2 NeuronCores.

Strategy: data-parallel over the batch (1024 rows/core); both embedding
tables are replicated in each core's HBM as one combined [2M, 128]
**bfloat16** table (v rows at +1M). Each partition owns 8 batch rows laid
out as 56 gather slots (8 u + 8 v + 40 neg). Five indirect-DMA gathers
(two sequence positions each) fetch 112 rows per partition into five
dedicated SBUF regions. Padding indices point at table row 0, which the
host zeroes, so masking is free.

Why bf16: random-row gather cost on this part is ~1.39 ns per 512B
descriptor (f32 rows, 368 GB/s/core — descriptor-rate-bound), but 256B
descriptors get packetized by the SWDGE and run ~4.4x faster per row
(measured 22.5 us vs 99.8 us per iteration for the same 71680 rows).
fp8 (128B rows) collapses to 172 us/iter — 256B is the sweet spot.

Fold split: the 10 position-planes are summed into a [128, 7168] slot
accumulator. PE folds columns [0, W_PE) via identity-matmul accumulation
into PSUM (8 banks = 3968 f32); DVE folds columns [W_PE, 7168) with
tensor_tensor adds (bf16, 2x mode); ACT drains PSUM to SBUF (bf16 cast).
The DVE epilogue forms u.v dots and negative-sample scores per slot, the
ACT computes softplus terms (Exp+Ln; Softplus is absent from this
compiler's ACT tables) accumulated per partition; the host sums 8x128x2
partials and divides by B.

HW contract notes (verified by probes in this repo):
 - indirect_dma_start needs a flat 2-dim dst AP [128, n*D]; idx [128, n]
   int32 pairs p-major: dst[p, j*D:(j+1)*D] = table[idx[p, j]].
 - bounds_check + oob_is_err=False on the indirect path is BROKEN in this
   walrus build (the index vector is ignored entirely) — padding must
   fetch the zeroed row 0; OOB-skip designs don't work.
 - CCE compute_op on the indirect path is device-fatal; plain gathers +
   engine folds instead.
 - This walrus build allows at most ONE attached sync wait per
   instruction; raw bass with explicit wait_ge instructions sidesteps it.
 - Every DMA must carry a completion semaphore ("DGE must have sync
   info").
 - Measured steady-state (repeat-slope, K=1 vs K=17 queued executions):
   see test.py output; the f32 baseline design measured 117.4 us/iter.
"""
import os
import numpy as np

VOCAB = 1_000_000
D = 128
B = 8192
NEG = 5
L = 10
NCORES = 8
BPC = B // NCORES            # batch rows per core
T = BPC // 128               # tile slots per partition (8)
SLOTS = T + T + T * NEG      # 56 gather slots per partition
W = SLOTS * D                # 7168 acc columns per partition
NU = T * D                   # 1024: u (and v) block columns
NG = 5                       # gathers per iteration (2 positions each)
PPG = L // NG                # positions per gather (2)

_STATE = {}
LAST_EXEC_NS = None


def _build_program(repeat=1, w_pe=3968):
    import concourse.bass as bass
    import concourse.mybir as mybir
    from concourse.bass import AP, IndirectOffsetOnAxis

    f32, i32 = mybir.dt.float32, mybir.dt.int32
    bf16 = mybir.dt.bfloat16
    nc = bass.Bass()
    big = nc.dram_tensor("big", [2 * VOCAB, D], bf16, kind="ExternalInput")
    idx_in = nc.dram_tensor("idx", [128, L * SLOTS], i32, kind="ExternalInput")
    ruv_in = nc.dram_tensor("ruv", [128, T], f32, kind="ExternalInput")
    rn_in = nc.dram_tensor("rn", [128, T * NEG], f32, kind="ExternalInput")
    ident_in = nc.dram_tensor("ident", [128, 128], bf16, kind="ExternalInput")
    out = nc.dram_tensor("lp", [128, 2], f32, kind="ExternalOutput")

    NPE_T = max(0, (w_pe - 2 * NU) // (NEG * D))   # neg t-groups on PE
    assert w_pe in (0, 2 * NU + NPE_T * NEG * D), "w_pe must align to blocks"
    WDV = W - w_pe                                 # DVE fold width
    # PSUM bank column ranges within [0, w_pe)
    banks = []
    c = 0
    while c < w_pe:
        wb = min(512, w_pe - c)
        banks.append((c, wb))
        c += wb

    from contextlib import ExitStack
    ctx = ExitStack()
    with ctx:
        idx_t = ctx.enter_context(nc.sbuf_tensor([128, L * SLOTS], i32))
        regions = [
            ctx.enter_context(nc.sbuf_tensor(f"reg{g}", [128, PPG * W], bf16))
            for g in range(NG)
        ]
        accdve = ctx.enter_context(
            nc.sbuf_tensor([128, max(WDV, 1)], bf16))
        accpe = ctx.enter_context(
            nc.sbuf_tensor([128, max(w_pe, 1)], bf16))
        ident = ctx.enter_context(nc.sbuf_tensor([128, 128], bf16))
        ruv_t = ctx.enter_context(nc.sbuf_tensor([128, T], f32))
        rn_t = ctx.enter_context(nc.sbuf_tensor([128, T * NEG], f32))
        prod_uv = ctx.enter_context(nc.sbuf_tensor([128, NU], bf16))
        prod_n1 = ctx.enter_context(
            nc.sbuf_tensor([128, max(NPE_T * NEG * D, 1)], bf16))
        prod_n2 = ctx.enter_context(
            nc.sbuf_tensor([128, max((T - NPE_T) * NEG * D, 1)], bf16))
        score_raw = ctx.enter_context(nc.sbuf_tensor([128, T], f32))
        nscore_raw = ctx.enter_context(nc.sbuf_tensor([128, T * NEG], f32))
        score = ctx.enter_context(nc.sbuf_tensor([128, T], f32))
        nscore = ctx.enter_context(nc.sbuf_tensor([128, T * NEG], f32))
        plt_t = ctx.enter_context(nc.sbuf_tensor([128, T], f32))
        nlt_t = ctx.enter_context(nc.sbuf_tensor([128, T * NEG], f32))
        lp_t = ctx.enter_context(nc.sbuf_tensor([128, 2], f32))
        psums = [
            ctx.enter_context(nc.psum_tensor(f"ps{i}", [128, wb], f32))
            for i, (c0, wb) in enumerate(banks)
        ]
        s_idx = ctx.enter_context(nc.semaphore("s_idx"))
        s_g = ctx.enter_context(nc.semaphore("s_g"))
        s_f = ctx.enter_context(nc.semaphore("s_f"))
        s_peg = ctx.enter_context(nc.semaphore("s_peg"))
        s_cp = ctx.enter_context(nc.semaphore("s_cp"))
        s_dve = ctx.enter_context(nc.semaphore("s_dve"))
        s_act = ctx.enter_context(nc.semaphore("s_act"))
        s_out = ctx.enter_context(nc.semaphore("s_out"))
        block = ctx.enter_context(nc.Block())

        def sub(g, h, c0, c1):
            """Columns [c0,c1) of position-plane h within region g."""
            return regions[g][:, h * W + c0:h * W + c1]

        @block.sync
        def _(sync):
            sync.dma_start(out=idx_t[:], in_=idx_in[:]).then_inc(s_idx, 16)
            sync.dma_start(out=ruv_t[:], in_=ruv_in[:]).then_inc(s_idx, 16)
            sync.dma_start(out=rn_t[:], in_=rn_in[:]).then_inc(s_idx, 16)
            sync.dma_start(out=ident[:], in_=ident_in[:]).then_inc(s_idx, 16)
            for r in range(repeat):
                sync.wait_ge(s_act, 2 * (r + 1))
                sync.dma_start(out=out[:], in_=lp_t[:]).then_inc(s_out, 16)

        @block.gpsimd
        def _(gpsimd):
            gpsimd.wait_ge(s_idx, 16)
            for r in range(repeat):
                for g in range(NG):
                    if r > 0:
                        # WAR: engines must have consumed this region's
                        # previous contents (PE groups 2g,2g+1; DVE folds)
                        if w_pe > 0:
                            gpsimd.wait_ge(s_peg, L * (r - 1) + 2 * g + 2)
                        if WDV > 0:
                            gpsimd.wait_ge(s_f, 9 * (r - 1) + max(2 * g + 1, 1))
                    gpsimd.indirect_dma_start(
                        out=regions[g][:],
                        out_offset=None,
                        in_=big[:],
                        in_offset=IndirectOffsetOnAxis(
                            ap=idx_t[:, PPG * SLOTS * g:PPG * SLOTS * (g + 1)],
                            axis=0,
                        ),
                    ).then_inc(s_g, 16)

        if w_pe > 0:
            @block.tensor
            def _(tensor):
                tensor.wait_ge(s_idx, 64)         # ident loaded
                for r in range(repeat):
                    if r > 0:
                        tensor.wait_ge(s_cp, r)   # WAR: psum drained
                    for l in range(L):
                        g, h = l // PPG, l % PPG
                        if h == 0:
                            tensor.wait_ge(s_g, 16 * (NG * r + g + 1))
                        last = None
                        for (c0, wb), ps in zip(banks, psums):
                            last = tensor.matmul(
                                out=ps[:],
                                lhsT=ident[:],
                                rhs=sub(g, h, c0, c0 + wb),
                                start=(l == 0), stop=(l == L - 1),
                                skip_group_check=True,
                            )
                        last.then_inc(s_peg, 1)

            @block.scalar
            def _(scalar):
                Copy = mybir.ActivationFunctionType.Copy
                for r in range(repeat):
                    scalar.wait_ge(s_peg, L * (r + 1))
                    if r > 0:
                        scalar.wait_ge(s_dve, 2 * r)  # WAR: accpe consumed
                    last = None
                    for (c0, wb), ps in zip(banks, psums):
                        last = scalar.activation(
                            out=accpe[:, c0:c0 + wb], in_=ps[:], func=Copy,
                        )
                    last.then_inc(s_cp, 1)
                    # softplus(x) = ln(1 + exp(x)); Softplus itself is
                    # absent from this compiler's ACT table set.
                    scalar.wait_ge(s_dve, 2 * (r + 1))
                    if r > 0:
                        scalar.wait_ge(s_out, 16 * r)
                    scalar.activation(
                        out=plt_t[:], in_=score[:],
                        func=mybir.ActivationFunctionType.Exp, scale=-1.0,
                    )
                    scalar.activation(
                        out=plt_t[:], in_=plt_t[:],
                        func=mybir.ActivationFunctionType.Ln,
                        bias=1.0, accum_out=lp_t[:, 0:1],
                    ).then_inc(s_act, 1)
                    scalar.activation(
                        out=nlt_t[:], in_=nscore[:],
                        func=mybir.ActivationFunctionType.Exp, scale=1.0,
                    )
                    scalar.activation(
                        out=nlt_t[:], in_=nlt_t[:],
                        func=mybir.ActivationFunctionType.Ln,
                        bias=1.0, accum_out=lp_t[:, 1:2],
                    ).then_inc(s_act, 1)
        else:
            @block.scalar
            def _(scalar):
                for r in range(repeat):
                    scalar.wait_ge(s_dve, 2 * (r + 1))
                    if r > 0:
                        scalar.wait_ge(s_out, 16 * r)
                    scalar.activation(
                        out=plt_t[:], in_=score[:],
                        func=mybir.ActivationFunctionType.Exp, scale=-1.0,
                    )
                    scalar.activation(
                        out=plt_t[:], in_=plt_t[:],
                        func=mybir.ActivationFunctionType.Ln,
                        bias=1.0, accum_out=lp_t[:, 0:1],
                    ).then_inc(s_act, 1)
                    scalar.activation(
                        out=nlt_t[:], in_=nscore[:],
                        func=mybir.ActivationFunctionType.Exp, scale=1.0,
                    )
                    scalar.activation(
                        out=nlt_t[:], in_=nlt_t[:],
                        func=mybir.ActivationFunctionType.Ln,
                        bias=1.0, accum_out=lp_t[:, 1:2],
                    ).then_inc(s_act, 1)

        @block.vector
        def _(vector):
            for r in range(repeat):
                if WDV > 0:
                    # fold 1: acc = plane0 + plane1 (both halves of reg 0)
                    vector.wait_ge(s_g, 16 * (NG * r + 1))
                    vector.tensor_tensor(
                        out=accdve[:], in0=sub(0, 0, w_pe, W),
                        in1=sub(0, 1, w_pe, W), op=mybir.AluOpType.add,
                    ).then_inc(s_f, 1)
                    for l in range(2, L):
                        g, h = l // PPG, l % PPG
                        if h == 0:
                            vector.wait_ge(s_g, 16 * (NG * r + g + 1))
                        vector.tensor_tensor(
                            out=accdve[:], in0=accdve[:],
                            in1=sub(g, h, w_pe, W), op=mybir.AluOpType.add,
                        ).then_inc(s_f, 1)
                # ---- epilogue ----
                if w_pe > 0:
                    vector.wait_ge(s_cp, r + 1)
                # u/v blocks live in accpe when PE folds them, else accdve
                uv_t = accpe if w_pe >= 2 * NU else accdve
                ape = uv_t[:]
                adv = accdve[:]
                NPE_N = NPE_T * NEG * D
                neg2_off = 2 * NU + NPE_N - w_pe  # neg tail offset in accdve
                vector.tensor_tensor(
                    out=prod_uv[:], in0=uv_t[:, 0:NU], in1=uv_t[:, NU:2 * NU],
                    op=mybir.AluOpType.mult,
                )
                vector.tensor_reduce(
                    out=score_raw[:],
                    in_=prod_uv[:].rearrange("p (t d) -> p t d", d=D),
                    axis=mybir.AxisListType.X,
                    op=mybir.AluOpType.add,
                )
                if NPE_T > 0:
                    u_b1 = AP(
                        ape.tensor, ape.offset,
                        [[ape.ap[0][0], 128], [D, NPE_T], [0, NEG], [1, D]],
                    )
                    vector.tensor_tensor(
                        out=prod_n1[:],
                        in0=accpe[:, 2 * NU:2 * NU + NPE_N].rearrange(
                            "p (t n d) -> p t n d", n=NEG, d=D),
                        in1=u_b1,
                        op=mybir.AluOpType.mult,
                    )
                    vector.tensor_reduce(
                        out=nscore_raw[:, 0:NPE_T * NEG],
                        in_=prod_n1[:].rearrange("p (s d) -> p s d", d=D),
                        axis=mybir.AxisListType.X,
                        op=mybir.AluOpType.add,
                    )
                if NPE_T < T:
                    u_b2 = AP(
                        ape.tensor, ape.offset + NPE_T * D,
                        [[ape.ap[0][0], 128], [D, T - NPE_T], [0, NEG], [1, D]],
                    )
                    vector.tensor_tensor(
                        out=prod_n2[:],
                        in0=accdve[
                            :, neg2_off:neg2_off + (T - NPE_T) * NEG * D
                        ].rearrange("p (t n d) -> p t n d", n=NEG, d=D),
                        in1=u_b2,
                        op=mybir.AluOpType.mult,
                    )
                    vector.tensor_reduce(
                        out=nscore_raw[:, NPE_T * NEG:],
                        in_=prod_n2[:].rearrange("p (s d) -> p s d", d=D),
                        axis=mybir.AxisListType.X,
                        op=mybir.AluOpType.add,
                    )
                if r == 0:
                    vector.wait_ge(s_idx, 48)
                if r > 0:
                    vector.wait_ge(s_act, 2 * r)
                vector.tensor_tensor(
                    out=score[:], in0=score_raw[:], in1=ruv_t[:],
                    op=mybir.AluOpType.mult,
                ).then_inc(s_dve, 1)
                vector.tensor_tensor(
                    out=nscore[:], in0=nscore_raw[:], in1=rn_t[:],
                    op=mybir.AluOpType.mult,
                ).then_inc(s_dve, 1)

    return nc


def _prep_core_inputs(c, pos_u, pos_v, neg_v, lu_all, lv_all, ln_all):
    """Index tile [128, L*SLOTS] + recip tiles for core c."""
    bsel = slice(c * BPC, (c + 1) * BPC)
    nsel = slice(c * BPC * NEG, (c + 1) * BPC * NEG)
    pu = pos_u[bsel].reshape(T, 128, L).transpose(1, 2, 0)          # [p, l, t]
    pv = pos_v[bsel].reshape(T, 128, L).transpose(1, 2, 0)          # [p, l, t]
    nv = neg_v[nsel].reshape(T, 128, NEG, L).transpose(1, 3, 0, 2)  # [p, l, t, n]
    nv = nv.reshape(128, L, T * NEG)

    idx = np.empty((128, L, SLOTS), np.int64)
    idx[:, :, 0:T] = pu
    idx[:, :, T:2 * T] = np.where(pv == 0, 0, pv + VOCAB)
    idx[:, :, 2 * T:] = np.where(nv == 0, 0, nv + VOCAB)
    # padding (index 0) gathers the zeroed row 0 and contributes nothing
    idx_flat = np.ascontiguousarray(idx.reshape(128, L * SLOTS), dtype=np.int32)

    lu = lu_all[bsel].reshape(T, 128).T.astype(np.float64)           # [p, t]
    lv = lv_all[bsel].reshape(T, 128).T.astype(np.float64)
    ln = ln_all[nsel].reshape(T, 128, NEG).transpose(1, 0, 2).astype(np.float64)
    ruv = (1.0 / (lu * lv)).astype(np.float32)
    rn = (1.0 / (lu[:, :, None] * ln)).reshape(128, T * NEG).astype(np.float32)
    return idx_flat, np.ascontiguousarray(ruv), np.ascontiguousarray(rn)


def prep_inputs(u_table, v_table, pos_u, pos_u_lens, pos_v, pos_v_lens,
                neg_v, neg_v_lens):
    """Host-side prep: per-core small inputs + the shared big bf16 table."""
    import ml_dtypes

    u_table = np.asarray(u_table)
    v_table = np.asarray(v_table)
    pos_u = np.asarray(pos_u)
    pos_v = np.asarray(pos_v)
    neg_v = np.asarray(neg_v)
    pos_u_lens = np.asarray(pos_u_lens)
    pos_v_lens = np.asarray(pos_v_lens)
    neg_v_lens = np.asarray(neg_v_lens)

    big = np.empty((2 * VOCAB, D), ml_dtypes.bfloat16)
    big[:VOCAB] = u_table.astype(ml_dtypes.bfloat16)
    big[VOCAB:] = v_table.astype(ml_dtypes.bfloat16)
    big[0] = 0.0          # padding rows (index 0) must contribute zero
    big[VOCAB] = 0.0

    ident = np.eye(128, dtype=ml_dtypes.bfloat16)
    in_maps = []
    for c in range(NCORES):
        idx, ruv, rn = _prep_core_inputs(
            c, pos_u, pos_v, neg_v, pos_u_lens, pos_v_lens, neg_v_lens
        )
        in_maps.append({"idx": idx, "ruv": ruv, "rn": rn, "ident": ident})
    return in_maps, big


def kernel(u_table, v_table, pos_u, pos_u_lens, pos_v, pos_v_lens,
           neg_v, neg_v_lens):
    global LAST_EXEC_NS
    from concourse.bass_utils import run_bass_kernel_spmd

    in_maps, big = prep_inputs(u_table, v_table, pos_u, pos_u_lens,
                               pos_v, pos_v_lens, neg_v, neg_v_lens)
    for m in in_maps:
        m["big"] = big

    w_pe = int(os.environ.get("KERNEL_WPE", "3968"))
    key = ("nc", w_pe)
    if key not in _STATE:
        _STATE[key] = _build_program(w_pe=w_pe)
    nc = _STATE[key]

    trace = bool(int(os.environ.get("KERNEL_TRACE", "0")))
    res = run_bass_kernel_spmd(
        nc, in_maps, core_ids=list(range(NCORES)), trace=trace,
    )
    LAST_EXEC_NS = res.exec_time_ns

    total = np.float64(0.0)
    for r in res.results:
        total += r["lp"].astype(np.float64).sum()
    return np.float32(total / B)
